# revision 1
# baseline (speedup 1.0000x reference)
"""GRU encoder with alive-sieve freeze on 8 Trainium2 cores.

Problem: utterance [M=128, N=1024] int32 tokens, emb_table [V=32000, E=512],
GRUCell with W_ih/W_hh [3E, E], biases [3E]. Rows freeze after the step where
their token == term_id. Output: final hidden state [N, E] f32.

Strategy: data-parallel over batch (128 rows/core, batch on SBUF partitions).
Per core, per time step:
  - emb_T obtained via dma_gather(transpose=True) from a bf16 copy of the
    table: out[p, c, t] = emb[tok_t, c*128+p] -> ready-to-use matmul lhsT.
  - gi = emb @ W_ih.T (+ biases via K=1 ones-row matmuls) accumulated in PSUM,
    prefetched one step ahead; gh = state @ W_hh.T accumulates into the same
    r/z PSUM banks (n-gate kept separate for r*h_n).
  - gates on ACT (sigmoid/tanh) + DVE; the alive-freeze folds into the final
    blend: state' = (f*u0)*alive + state  with u0 = sigmoid(-p_z) = 1-z,
    f = n - state, alive a per-partition scalar from a host-precomputed mask.
  - state' transposed on PE (bf16) to feed the next step's gh.
"""

import os

import numpy as np
import ml_dtypes

M, N, V, E = 128, 1024, 32000, 512
NCORES = 8
BS = N // NCORES          # batch rows per core
KCH = E // 128            # k-chunks of the contraction dim
GB = 4                    # time steps per gather block (512 idxs/gather: 1024 hits a SWDGE descriptor limit on HW)
E3 = 3 * E

TRACE = os.environ.get("GRU_TRACE", "0") == "1"
GP_TAIL = os.environ.get("GRU_GP_TAIL", "0") == "1"
ABLATE = os.environ.get("GRU_ABLATE", "")  # "", "pe", "chain"
LAST_RESULT = {}

_nc_cache = {}


def _build(n_steps, repeat=1):
    """repeat>1 wraps the whole GRU in an on-device For_i loop: a
    timing-only build that amortizes host/RPC overhead over `repeat`
    back-to-back executions of the full kernel body."""
    import contextlib

    import concourse.bacc as bacc
    import concourse.mybir as mybir
    import concourse.tile as tile
    from concourse.masks import make_identity

    dt = mybir.dt
    f32, bf16, i16 = dt.float32, dt.bfloat16, dt.int16
    AF = mybir.ActivationFunctionType
    OP = mybir.AluOpType

    nblk = (n_steps + GB - 1) // GB

    nc = bacc.Bacc("TRN2", target_bir_lowering=False, debug=False)

    emb = nc.dram_tensor("emb", [V, E], bf16, kind="ExternalInput")
    idx = nc.dram_tensor("idx", [128, M * BS // 16], i16, kind="ExternalInput")
    alive = nc.dram_tensor("alive", [BS, M], f32, kind="ExternalInput")
    wih = nc.dram_tensor("wih", [128, KCH, E3], bf16, kind="ExternalInput")
    whh = nc.dram_tensor("whh", [128, KCH, E3], bf16, kind="ExternalInput")
    brz = nc.dram_tensor("brz", [1, 2 * E], bf16, kind="ExternalInput")
    bin_ = nc.dram_tensor("bin", [1, E], bf16, kind="ExternalInput")
    bhn = nc.dram_tensor("bhn", [1, E], bf16, kind="ExternalInput")
    out = nc.dram_tensor("out", [BS, E], f32, kind="ExternalOutput")

    with tile.TileContext(nc) as tc:
        with (
            tc.tile_pool(name="const", bufs=1) as cp,
            tc.tile_pool(name="gath", bufs=3) as gp,
            tc.tile_pool(name="work", bufs=2) as wp,
            tc.tile_pool(name="ps", bufs=2, space="PSUM") as ps,
        ):
            # ---- resident constants ----
            wih_sb = cp.tile([128, KCH, E3], bf16)
            nc.sync.dma_start(wih_sb[:], wih[:])
            whh_sb = cp.tile([128, KCH, E3], bf16)
            nc.sync.dma_start(whh_sb[:], whh[:])
            brz_sb = cp.tile([1, 2 * E], bf16)
            nc.sync.dma_start(brz_sb[:], brz[:])
            bin_sb = cp.tile([1, E], bf16)
            nc.sync.dma_start(bin_sb[:], bin_[:])
            bhn_sb = cp.tile([1, E], bf16)
            nc.sync.dma_start(bhn_sb[:], bhn[:])
            alive_sb = cp.tile([BS, M], f32)
            nc.sync.dma_start(alive_sb[:], alive[:])
            idx_sb = cp.tile([128, M * BS // 16], i16)
            nc.sync.dma_start(idx_sb[:], idx[:])
            ones_sb = cp.tile([1, 128], bf16)
            nc.vector.memset(ones_sb[:], 1.0)
            ident = cp.tile([128, 128], bf16)
            make_identity(nc, ident[:])

            rep_cm = tc.For_i(0, repeat, 1) if repeat > 1 \
                else contextlib.nullcontext()
            with rep_cm:
                _body(nc, tc, cp, gp, wp, ps, n_steps, locals())

    nc.compile()
    return nc


def _body(nc, tc, cp, gp, wp, ps, n_steps, env):
    import concourse.mybir as mybir
    dt = mybir.dt
    f32, bf16 = dt.float32, dt.bfloat16
    AF = mybir.ActivationFunctionType
    OP = mybir.AluOpType
    wih_sb, whh_sb = env["wih_sb"], env["whh_sb"]
    brz_sb, bin_sb, bhn_sb = env["brz_sb"], env["bin_sb"], env["bhn_sb"]
    alive_sb, idx_sb = env["alive_sb"], env["idx_sb"]
    ones_sb, ident = env["ones_sb"], env["ident"]
    emb, out = env["emb"], env["out"]
    nblk = (n_steps + GB - 1) // GB
    if True:
        if True:
            # ---- initial state (zeros) ----
            state = wp.tile([BS, E], f32, tag="state")
            nc.vector.memset(state[:], 0.0)
            stT = wp.tile([128, E], bf16, tag="stT")
            nc.vector.memset(stT[:], 0.0)

            # ---- gather blocks (prefetched) ----
            emb_blocks = [None] * nblk

            def issue_gather(g):
                et = gp.tile([128, KCH, GB * BS], bf16, tag="embT",
                             name=f"embT_{g}")
                cols = GB * BS // 16
                nc.gpsimd.dma_gather(
                    et[:], emb[:], idx_sb[:, g * cols:(g + 1) * cols],
                    num_idxs=GB * BS, num_idxs_reg=GB * BS, elem_size=E,
                    transpose=True,
                )
                emb_blocks[g] = et

            def gi_phase(t):
                """Emit bias + input-side matmuls for step t into fresh PSUM
                tiles. Returns (Pr, Pz, Pin, Phn)."""
                Pr = ps.tile([BS, E], f32, tag="pr", name=f"pr_{t}")
                Pz = ps.tile([BS, E], f32, tag="pz", name=f"pz_{t}")
                Pin = ps.tile([BS, E], f32, tag="pin", name=f"pin_{t}")
                Phn = ps.tile([BS, E], f32, tag="phnT", name=f"phn_{t}")
                nc.tensor.matmul(Pr[:], ones_sb[:], brz_sb[:, 0:E],
                                 start=True, stop=False)
                nc.tensor.matmul(Pz[:], ones_sb[:], brz_sb[:, E:2 * E],
                                 start=True, stop=False)
                nc.tensor.matmul(Pin[:], ones_sb[:], bin_sb[:],
                                 start=True, stop=False)
                nc.tensor.matmul(Phn[:], ones_sb[:], bhn_sb[:],
                                 start=True, stop=False)
                if ABLATE != "chain":
                    et = emb_blocks[t // GB]
                    s = (t % GB) * BS
                    for k in range(KCH):
                        lhs = et[:, k, s:s + BS]
                        nc.tensor.matmul(Pr[:], lhs, wih_sb[:, k, 0:E],
                                         start=False, stop=False)
                        nc.tensor.matmul(Pz[:], lhs, wih_sb[:, k, E:2 * E],
                                         start=False, stop=False)
                        nc.tensor.matmul(Pin[:], lhs, wih_sb[:, k, 2 * E:E3],
                                         start=False, stop=(k == KCH - 1))
                return Pr, Pz, Pin, Phn

            issue_gather(0)
            if nblk > 1:
                issue_gather(1)
            cur = gi_phase(0)

            EH = E // 2          # half of the hidden dim
            H0 = slice(0, EH)
            H1 = slice(EH, E)

            for t in range(n_steps):
                Pr, Pz, Pin, Phn = cur

                # ---- gh: recurrent matmuls. r first (feeds sigmoid), then
                # hn in halves (lets d/e/tanh start on half 0 early), z last.
                for k in range(KCH if ABLATE != "chain" else 0):
                    nc.tensor.matmul(Pr[:], stT[:, k * 128:(k + 1) * 128],
                                     whh_sb[:, k, 0:E],
                                     start=False, stop=(k == KCH - 1))
                for h in ((H0, H1) if ABLATE != "chain" else ()):
                    for k in range(KCH):
                        nc.tensor.matmul(
                            Phn[:, h], stT[:, k * 128:(k + 1) * 128],
                            whh_sb[:, k, 2 * E + h.start:2 * E + h.stop],
                            start=False, stop=(k == KCH - 1 and h is H1),
                        )
                for k in range(KCH if ABLATE != "chain" else 0):
                    nc.tensor.matmul(Pz[:], stT[:, k * 128:(k + 1) * 128],
                                     whh_sb[:, k, E:2 * E],
                                     start=False, stop=(k == KCH - 1))

                # ---- prefetch: gather two blocks ahead, gi one step ahead
                if t % GB == 0 and t // GB + 2 < nblk:
                    issue_gather(t // GB + 2)
                if t + 1 < n_steps:
                    cur = gi_phase(t + 1)

                if ABLATE == "pe":
                    continue
                # ---- gates; h0 of the f/q/s' tail on DVE, h1 on GPSIMD ----
                r_sb = wp.tile([BS, E], f32, tag="r_sb", name=f"r_{t}")
                u0_sb = wp.tile([BS, E], f32, tag="u0_sb", name=f"u0_{t}")
                d_sb = wp.tile([BS, E], f32, tag="d_sb", name=f"d_{t}")
                e_sb = wp.tile([BS, E], f32, tag="e_sb", name=f"e_{t}")
                n_sb = wp.tile([BS, E], f32, tag="n_sb", name=f"n_{t}")
                f_sb = wp.tile([BS, E], f32, tag="f_sb", name=f"f_{t}")
                q_sb = wp.tile([BS, E], f32, tag="q_sb", name=f"q_{t}")
                state_new = wp.tile([BS, E], f32, tag="state", name=f"st_{t}")
                a_col = alive_sb[:, t:t + 1]

                # ACT stream: sr0, sr1, tanh0, sz0, tanh1, sz1
                nc.scalar.activation(r_sb[:, H0], Pr[:, H0], AF.Sigmoid)
                nc.scalar.activation(r_sb[:, H1], Pr[:, H1], AF.Sigmoid)
                # DVE stream: d0 e0 d1 e1 f0 q0 s0 ...
                nc.vector.tensor_tensor(d_sb[:, H0], r_sb[:, H0], Phn[:, H0],
                                        op=OP.mult)
                nc.vector.tensor_tensor(e_sb[:, H0], d_sb[:, H0], Pin[:, H0],
                                        op=OP.add)
                nc.scalar.activation(n_sb[:, H0], e_sb[:, H0], AF.Tanh)
                nc.scalar.activation(u0_sb[:, H0], Pz[:, H0], AF.Sigmoid,
                                     scale=-1.0)
                nc.vector.tensor_tensor(d_sb[:, H1], r_sb[:, H1], Phn[:, H1],
                                        op=OP.mult)
                nc.vector.tensor_tensor(e_sb[:, H1], d_sb[:, H1], Pin[:, H1],
                                        op=OP.add)
                nc.scalar.activation(n_sb[:, H1], e_sb[:, H1], AF.Tanh)
                nc.scalar.activation(u0_sb[:, H1], Pz[:, H1], AF.Sigmoid,
                                     scale=-1.0)
                # tail half 0 on DVE
                nc.vector.tensor_tensor(f_sb[:, H0], n_sb[:, H0],
                                        state[:, H0], op=OP.subtract)
                nc.vector.tensor_tensor(q_sb[:, H0], f_sb[:, H0],
                                        u0_sb[:, H0], op=OP.mult)
                nc.vector.scalar_tensor_tensor(
                    state_new[:, H0], q_sb[:, H0], a_col, state[:, H0],
                    op0=OP.mult, op1=OP.add)
                # tail half 1 (GP_TAIL picks GPSIMD vs DVE; blend on DVE:
                # TensorScalarPtr is not a Pool-engine opcode)
                eng1 = nc.gpsimd if GP_TAIL else nc.vector
                eng1.tensor_tensor(f_sb[:, H1], n_sb[:, H1],
                                   state[:, H1], op=OP.subtract)
                eng1.tensor_tensor(q_sb[:, H1], f_sb[:, H1],
                                   u0_sb[:, H1], op=OP.mult)
                nc.vector.scalar_tensor_tensor(
                    state_new[:, H1], q_sb[:, H1], a_col, state[:, H1],
                    op0=OP.mult, op1=OP.add)
                state = state_new

                # ---- transpose state for next step's gh ----
                if t + 1 < n_steps:
                    st_bf = wp.tile([BS, E], bf16, tag="st_bf", name=f"sb_{t}")
                    nc.vector.tensor_copy(st_bf[:, H0], state[:, H0])
                    eng1.tensor_copy(st_bf[:, H1], state[:, H1])
                    stT_ps = ps.tile([128, E], bf16, tag="phnT",
                                     name=f"stTp_{t}")
                    for c in range(KCH):
                        nc.tensor.transpose(
                            stT_ps[:, c * 128:(c + 1) * 128],
                            st_bf[:, c * 128:(c + 1) * 128], ident[:],
                        )
                    stT_new = wp.tile([128, E], bf16, tag="stT",
                                      name=f"stT_{t}")
                    nc.vector.tensor_copy(stT_new[:, H0], stT_ps[:, H0])
                    nc.vector.tensor_copy(stT_new[:, H1], stT_ps[:, H1])
                    stT = stT_new

            nc.sync.dma_start(out[:], state[:])


def _get_nc(n_steps, repeat=1):
    key = (n_steps, repeat)
    if key not in _nc_cache:
        _nc_cache[key] = _build(n_steps, repeat)
    return _nc_cache[key]


def _prep_inputs(utterance, emb_table, W_ih, W_hh, b_ih, b_hh, term_id):
    """Host-side sharding/layout prep. Returns per-core in_maps."""
    utterance = np.asarray(utterance, dtype=np.int32)
    emb_table = np.asarray(emb_table, dtype=np.float32)
    W_ih = np.asarray(W_ih, dtype=np.float32)
    W_hh = np.asarray(W_hh, dtype=np.float32)
    b_ih = np.asarray(b_ih, dtype=np.float32)
    b_hh = np.asarray(b_hh, dtype=np.float32)
    term = int(np.asarray(term_id))

    bf = ml_dtypes.bfloat16
    emb_bf = np.ascontiguousarray(emb_table.astype(bf))

    def wprep(W):  # [3E, E] -> [128, KCH, 3E] with w[p,k,n] = W[n, k*128+p]
        Wt = W.T.reshape(KCH, 128, E3).transpose(1, 0, 2)
        return np.ascontiguousarray(Wt.astype(bf))

    wih_h = wprep(W_ih)
    whh_h = wprep(W_hh)
    brz_h = np.ascontiguousarray(
        (b_ih[:2 * E] + b_hh[:2 * E]).reshape(1, 2 * E).astype(bf))
    bin_h = np.ascontiguousarray(b_ih[2 * E:].reshape(1, E).astype(bf))
    bhn_h = np.ascontiguousarray(b_hh[2 * E:].reshape(1, E).astype(bf))

    in_maps = []
    for c in range(NCORES):
        U = utterance[:, c * BS:(c + 1) * BS]          # [M, BS], (t, b)
        flat = U.reshape(-1).astype(np.int16)           # i = t*BS + b
        idx_h = np.ascontiguousarray(np.tile(flat.reshape(-1, 16).T, (8, 1)))  # [128, M*BS/16]
        hit = (U == term)
        csum = np.cumsum(hit, axis=0)
        aliveT = np.ones((M, BS), dtype=np.float32)
        aliveT[1:] = (csum[:-1] == 0)
        alive_h = np.ascontiguousarray(aliveT.T)        # [BS, M]
        in_maps.append({
            "emb": emb_bf, "idx": idx_h, "alive": alive_h,
            "wih": wih_h, "whh": whh_h,
            "brz": brz_h, "bin": bin_h, "bhn": bhn_h,
        })
    return in_maps


def kernel(utterance, emb_table, W_ih, W_hh, b_ih, b_hh, term_id,
           n_steps=M):
    from concourse.bass_utils import run_bass_kernel_spmd

    nc = _get_nc(n_steps)
    in_maps = _prep_inputs(utterance, emb_table, W_ih, W_hh, b_ih, b_hh,
                           term_id)
    res = run_bass_kernel_spmd(nc, in_maps, core_ids=list(range(NCORES)),
                               trace=TRACE)
    LAST_RESULT["exec_time_ns"] = res.exec_time_ns
    LAST_RESULT["trace"] = res.instructions_and_trace
    return np.concatenate([r["out"] for r in res.results], axis=0)

